# revision 1
# baseline (speedup 1.0000x reference)
"""CNN-LSTM Trainium2 kernel (nn_CNN_LSTM_41205916238256).

Per core (batch-parallel, 32 batch elems per core):
  Prologue, processed in four 127-step time-quarters whose instruction
  emission is interleaved into the LSTM recurrence (quarter q+1 is
  computed on otherwise-idle engines while the recurrence consumes
  quarter q):
    1. Embedding gather via indirect DMA per (batch, quarter): 128-token
       window + one shared 3-token straggler gather (conv halo).
    2. PE transposes (batched 4 per PSUM bank) -> embTq [E, 131 tok] bf16,
       copies on GPSIMD.
    3. Conv as 10 accumulated 127-col matmuls per (nf-half, batch);
       ReLU+bias on GPSIMD -> convq [nf, t*32+b] bf16.
    4. Xp = W_ih_eff @ conv (+bias on GPSIMD), stored per-quarter
       [128, t*128 + ch*64 + g*16 + b] bf16.
  Recurrence: two independent interleaved 16-batch chains, H on
  partitions, per chain and step:
    bank = eye@Xp_t (inject, start) + U_eff @ r_{t-1} (4 MMs)
    S    = sigmoid(bank)          [f,i,g,o at cols 0/16/32/48]   (ACT)
    t1   = (S_g - .5) * S_i                                      (DVE)
    t2   = S_f * Q                                               (DVE)
    Q'   = 2*t1 + t2        (Q == c == 2P)                       (DVE)
    sigP = tanh(Q')                                              (ACT)
    r    = sigP * S_o  (bf16)                                    (DVE)
  with g-row prescale x2 folded into weights; h_n = sigP*S_o in fp32.
"""
from collections import deque

import numpy as np
import ml_dtypes

import concourse.bacc as bacc
import concourse.bass as bass
import concourse.mybir as mybir
import concourse.tile as tile
from concourse.bass_utils import run_bass_kernel_spmd

BF16 = mybir.dt.bfloat16
F32 = mybir.dt.float32
I32 = mybir.dt.int32
AF = mybir.ActivationFunctionType
OP = mybir.AluOpType

VOCAB, EMB, KER, NF, HID = 50257, 256, 5, 256, 128
B, S = 256, 512
T = S - KER + 1            # 508
NC = 8                     # cores
BL = B // NC               # 32 batch per core
P = 128
QT = T // 4                # 127 timesteps per quarter
QW = QT + KER - 1          # 131-token window per quarter
CW = 16                    # chain width (batch per chain)
NCHAIN = 2

_PROGRAM = None


def _build_program():
    nc = bacc.Bacc("TRN2", target_bir_lowering=False, debug=False)

    emb_d = nc.dram_tensor("embt", [VOCAB, EMB], BF16, kind="ExternalInput")
    idxa_d = nc.dram_tensor("idxa", [P, BL * 4], I32, kind="ExternalInput")
    idxb_d = nc.dram_tensor("idxb", [P, 4], I32, kind="ExternalInput")
    cw_d = nc.dram_tensor("cw", [KER * 2 * 2, P, P], BF16, kind="ExternalInput")
    cb_d = nc.dram_tensor("cb", [P, 2], F32, kind="ExternalInput")
    wih_d = nc.dram_tensor("wih", [4 * 2, P, P], BF16, kind="ExternalInput")
    be_d = nc.dram_tensor("be", [P, 4], F32, kind="ExternalInput")
    u_d = nc.dram_tensor("u", [4, P, P], BF16, kind="ExternalInput")
    eye_d = nc.dram_tensor("eye", [P, P], BF16, kind="ExternalInput")
    r0_d = nc.dram_tensor("r0", [P, BL], BF16, kind="ExternalInput")
    hT_d = nc.dram_tensor("hT", [P, BL], F32, kind="ExternalOutput")

    with tile.TileContext(nc) as tc:
        with tc.tile_pool(name="stat", bufs=1) as stat, \
             tc.tile_pool(name="pgat", bufs=24) as pg, \
             tc.tile_pool(name="pemb", bufs=2) as pe, \
             tc.tile_pool(name="pconv", bufs=2) as pc, \
             tc.tile_pool(name="pxp", bufs=3) as px, \
             tc.tile_pool(name="ptr", bufs=1, space="PSUM") as ptr, \
             tc.tile_pool(name="pcps", bufs=1, space="PSUM") as pcps, \
             tc.tile_pool(name="pxps", bufs=2, space="PSUM") as pxps, \
             tc.tile_pool(name="rdyn", bufs=3) as dyn, \
             tc.tile_pool(name="rps", bufs=2, space="PSUM") as rps:
            # ---- static loads
            idxa_t = stat.tile([P, BL * 4], I32, tag="idxa")
            nc.sync.dma_start(out=idxa_t[:], in_=idxa_d[:])
            idxb_t = stat.tile([P, 4], I32, tag="idxb")
            nc.sync.dma_start(out=idxb_t[:], in_=idxb_d[:])
            cw_t = []
            for k in range(KER):
                for eh in range(2):
                    for nh in range(2):
                        w = stat.tile([P, P], BF16, tag=f"cw{k}{eh}{nh}")
                        nc.sync.dma_start(out=w[:], in_=cw_d[(k * 2 + eh) * 2 + nh])
                        cw_t.append(w)
            cwf = lambda k, eh, nh: cw_t[(k * 2 + eh) * 2 + nh]
            cb_t = stat.tile([P, 2], F32, tag="cb")
            nc.sync.dma_start(out=cb_t[:], in_=cb_d[:])
            wih_t = []
            for g in range(4):
                for kh in range(2):
                    w = stat.tile([P, P], BF16, tag=f"wih{g}{kh}")
                    nc.sync.dma_start(out=w[:], in_=wih_d[g * 2 + kh])
                    wih_t.append(w)
            be_t = stat.tile([P, 4], F32, tag="be")
            nc.sync.dma_start(out=be_t[:], in_=be_d[:])
            u_t = []
            for g in range(4):
                w = stat.tile([P, P], BF16, tag=f"u{g}")
                nc.sync.dma_start(out=w[:], in_=u_d[g])
                u_t.append(w)
            eye_t = stat.tile([P, P], BF16, tag="eye")
            nc.sync.dma_start(out=eye_t[:], in_=eye_d[:])
            r0_t = stat.tile([P, BL], BF16, tag="r0")
            nc.sync.dma_start(out=r0_t[:], in_=r0_d[:])
            zeros_t = stat.tile([P, 512], F32, tag="zeros")
            nc.vector.memset(zeros_t[:], 0.0)

            # ---------- prologue emitters (per time-quarter) ----------
            embTq = {}     # qi -> [eh][P, BL*QW] bf16
            convq = {}     # qi -> [nh][P, QT*BL] bf16, col = t*32 + b
            xpq = {}       # qi -> [P, QT*128] bf16, col = t*128+ch*64+g*16+b

            def em_gather4(qi, bg):
                # gather 4 batch elems' 128-token windows
                Gs = []
                for bi in range(4):
                    c = qi * BL + bg * 4 + bi
                    G = pg.tile([P, EMB], BF16, tag="G", name="G")
                    nc.gpsimd.indirect_dma_start(
                        out=G[:], out_offset=None, in_=emb_d[:],
                        in_offset=bass.IndirectOffsetOnAxis(
                            ap=idxa_t[:, c:c + 1], axis=0))
                    Gs.append(G)
                return Gs

            def em_transpose4(qi, bg, Gs):
                # 8 transposes into one PSUM bank, 2 batched copies out
                tp = ptr.tile([P, 1024], BF16, tag="tp", name="tp")
                for bi in range(4):
                    for eh in range(2):
                        nc.tensor.transpose(
                            out=tp[:, (bi * 2 + eh) * P:(bi * 2 + eh) * P + P],
                            in_=Gs[bi][:, eh * P:eh * P + P],
                            identity=eye_t[:])
                tpv = tp[:].rearrange("p (b e c) -> p b e c", b=4, e=2, c=P)
                ev = embTq[qi]
                for eh in range(2):
                    dst = ev[eh][:].rearrange("p (b w) -> p b w", b=BL, w=QW)
                    nc.vector.tensor_copy(
                        out=dst[:, bg * 4:bg * 4 + 4, 0:P],
                        in_=tpv[:, :, eh, :])

            def em_straggler(qi):
                # one gather covering rows b*4+j (j<3): tokens 127q+128+j
                G2 = pg.tile([P, EMB], BF16, tag="G2", name="G2")
                nc.gpsimd.indirect_dma_start(
                    out=G2[:], out_offset=None, in_=emb_d[:],
                    in_offset=bass.IndirectOffsetOnAxis(
                        ap=idxb_t[:, qi:qi + 1], axis=0))
                tp = ptr.tile([P, 1024], BF16, tag="tp", name="tp")
                for eh in range(2):
                    nc.tensor.transpose(
                        out=tp[:, eh * P:eh * P + P],
                        in_=G2[:, eh * P:eh * P + P], identity=eye_t[:])
                tpv = tp[:, 0:256].rearrange("p (e b j) -> p e b j", e=2, b=BL, j=4)
                ev = embTq[qi]
                for eh in range(2):
                    dst = ev[eh][:].rearrange("p (b w) -> p b w", b=BL, w=QW)
                    nc.vector.tensor_copy(
                        out=dst[:, :, P:P + 3], in_=tpv[:, eh, :, 0:3])

            def em_conv_mm(qi, nh, bg, pair, cps):
                # 5 accumulated MMs (one per tap over both E-halves is 10;
                # emit 5 here per sub-thunk), each covering 4 batch elems
                ev_v = [embTq[qi][eh][:].rearrange("p (b w) -> p b w",
                                                   b=BL, w=QW)
                        for eh in range(2)]
                cpv = cps[:].rearrange("p (b t) -> p b t", b=4, t=QT)
                idx = 0
                for k in range(KER):
                    for eh in range(2):
                        if idx // 5 == pair:
                            nc.tensor.matmul(
                                out=cpv,
                                lhsT=cwf(k, eh, nh)[:],
                                rhs=ev_v[eh][:, bg * 4:bg * 4 + 4, k:k + QT],
                                start=(idx == 0), stop=(idx == 9))
                        idx += 1

            def em_conv_relu(qi, nh, bg, cps):
                cv = convq[qi][nh][:].rearrange("p (t b) -> p t b", t=QT, b=BL)
                cpv = cps[:].rearrange("p (b t) -> p t b", b=4, t=QT)
                nc.vector.scalar_tensor_tensor(
                    out=cv[:, :, bg * 4:bg * 4 + 4], in0=cpv,
                    scalar=cb_t[:, nh:nh + 1], in1=zeros_t[:, 0:4 * QT].rearrange(
                        "p (t b) -> p t b", t=QT, b=4),
                    op0=OP.add, op1=OP.max)

            def em_xp(qi, g, blk):
                tl0 = blk * 32
                L = min(32, QT - tl0)
                xv = xpq[qi][:].rearrange("p (t c g b) -> p t c g b",
                                          t=QT, c=NCHAIN, g=4, b=CW)
                cvv = [convq[qi][kh][:].rearrange("p (t b) -> p t b",
                                                  t=QT, b=BL)
                       for kh in range(2)]
                for ch in range(NCHAIN):
                    xps = pxps.tile([P, 512], F32, tag="xps", name="xps")
                    for kh in range(2):
                        nc.tensor.matmul(
                            out=xps[:, 0:L * CW], lhsT=wih_t[g * 2 + kh][:],
                            rhs=cvv[kh][:, tl0:tl0 + L, ch * CW:(ch + 1) * CW],
                            start=(kh == 0), stop=(kh == 1))
                    xo = xv[:, tl0:tl0 + L, ch, g, :]
                    xi = xps[:, 0:L * CW].rearrange("p (t b) -> p t b",
                                                    t=L, b=CW)
                    nc.scalar.activation(xo, xi, AF.Identity,
                                         bias=be_t[:, g:g + 1])

            def quarter_parts(qi):
                embTq[qi] = [pe.tile([P, BL * QW], BF16, tag=f"embT{eh}",
                                     name=f"embT{eh}") for eh in range(2)]
                convq[qi] = [pc.tile([P, QT * BL], BF16, tag=f"convT{nh}",
                                     name=f"convT{nh}") for nh in range(2)]
                xpq[qi] = px.tile([P, QT * P], BF16, tag="xpq", name="xpq")
                Gs_box = {}
                gth, tth, cth, xth = [], [], [], []
                for bg in range(8):
                    def gt(qi=qi, bg=bg):
                        Gs_box[bg] = em_gather4(qi, bg)
                    gth.append(gt)
                for bg in range(8):
                    def tt(qi=qi, bg=bg):
                        em_transpose4(qi, bg, Gs_box[bg])
                    tth.append(tt)
                    if bg == 0:
                        tth.append(lambda qi=qi: em_straggler(qi))
                for nh in range(2):
                    for bg in range(8):
                        box = {}
                        def mkcps(qi=qi, nh=nh, bg=bg, box=box):
                            pool, tg = ((pcps, "cps") if (nh * 8 + bg) % 2 == 0
                                        else (pxps, "xps"))
                            box["cps"] = pool.tile([P, 4 * QT], F32,
                                                   tag=tg, name="cps")
                        for pair in range(2):
                            def cm(qi=qi, nh=nh, bg=bg, pair=pair, box=box,
                                   mk=mkcps if pair == 0 else None):
                                if mk is not None:
                                    mk()
                                em_conv_mm(qi, nh, bg, pair, box["cps"])
                            cth.append(cm)
                        cth.append(lambda qi=qi, nh=nh, bg=bg, box=box:
                                   em_conv_relu(qi, nh, bg, box["cps"]))
                for blk in range(4):
                    for g in range(4):
                        xth.append(lambda qi=qi, g=g, blk=blk: em_xp(qi, g, blk))
                return gth, tth, cth, xth

            # ---------- pre-phase: quarter 0 fully emitted ----------
            g0, t0_, c0, x0 = quarter_parts(0)
            for th in g0 + t0_ + c0 + x0:
                th()
            # deadline-aware schedule: step -> thunks
            sched = {}

            def place(thunks, lo, hi):
                n = len(thunks)
                for i, th in enumerate(thunks):
                    s = lo + (i * (hi - lo)) // max(1, n - 1) if n > 1 else lo
                    sched.setdefault(min(s, hi), []).append(th)

            g1, t1_, c1, x1 = quarter_parts(1)
            g2, t2_, c2, x2 = quarter_parts(2)
            g3, t3_, c3, x3 = quarter_parts(3)
            place(g1, 0, 7)
            place(t1_, 8, 32)
            place(c1, 33, 80)
            place(x1, 81, 96)       # deadline 127
            place(g2, 97, 104)
            place(t2_, 105, 129)
            place(c2, 130, 177)
            place(x2, 178, 193)     # deadline 254
            place(g3, 194, 201)
            place(t3_, 202, 226)
            place(c3, 227, 274)
            place(x3, 275, 291)     # deadline 381

            # ---------- recurrence ----------
            P_prev, r_prev, S_last, sigP_last = [], [], [None, None], [None, None]
            for ch in range(NCHAIN):
                pz = stat.tile([P, CW], F32, tag=f"Pinit{ch}", name=f"Pinit{ch}")
                nc.vector.memset(pz[:], 0.0)
                P_prev.append(pz)
                r_prev.append(r0_t[:, ch * CW:(ch + 1) * CW])
            for t in range(T):
                qi, tl = divmod(t, QT)
                bank, S_t, t1, t2, P_new, sigP, r_new = \
                    [[None, None] for _ in range(7)]
                for ch in range(NCHAIN):
                    bank[ch] = rps.tile([P, 4 * CW], F32, tag=f"bank{ch}",
                                        name=f"bank{ch}")
                    base = tl * P + ch * 4 * CW
                    nc.tensor.matmul(out=bank[ch][:], lhsT=eye_t[:],
                                     rhs=xpq[qi][:, base:base + 4 * CW],
                                     start=True, stop=False)
                    for g in range(4):
                        nc.tensor.matmul(out=bank[ch][:, g * CW:(g + 1) * CW],
                                         lhsT=u_t[g][:], rhs=r_prev[ch][:],
                                         start=False, stop=(g == 3))
                for ch in range(NCHAIN):
                    S_t[ch] = dyn.tile([P, 4 * CW], F32, tag=f"S{ch}",
                                       name=f"S{ch}")
                    nc.scalar.activation(S_t[ch][:], bank[ch][:], AF.Sigmoid)
                for ch in range(NCHAIN):
                    t1[ch] = dyn.tile([P, CW], F32, tag=f"t1{ch}", name=f"t1{ch}")
                    nc.vector.scalar_tensor_tensor(
                        out=t1[ch][:], in0=S_t[ch][:, 2 * CW:3 * CW], scalar=0.5,
                        in1=S_t[ch][:, CW:2 * CW],
                        op0=OP.subtract, op1=OP.mult)
                    t2[ch] = dyn.tile([P, CW], F32, tag=f"t2{ch}", name=f"t2{ch}")
                    nc.vector.tensor_tensor(out=t2[ch][:], in0=S_t[ch][:, 0:CW],
                                            in1=P_prev[ch][:], op=OP.mult)
                    # Q-state: Q = 2P = c;  Q' = 2*t1 + Sf*Q
                    P_new[ch] = dyn.tile([P, CW], F32, tag=f"Pn{ch}",
                                         name=f"Pn{ch}")
                    nc.vector.scalar_tensor_tensor(
                        out=P_new[ch][:], in0=t1[ch][:], scalar=2.0, in1=t2[ch][:],
                        op0=OP.mult, op1=OP.add)
                for ch in range(NCHAIN):
                    sigP[ch] = dyn.tile([P, CW], F32, tag=f"sigP{ch}",
                                        name=f"sigP{ch}")
                    nc.scalar.activation(sigP[ch][:], P_new[ch][:], AF.Tanh)
                for ch in range(NCHAIN):
                    r_new[ch] = dyn.tile([P, CW], BF16, tag=f"r{ch}", name=f"r{ch}")
                    nc.vector.tensor_tensor(out=r_new[ch][:], in0=sigP[ch][:],
                                            in1=S_t[ch][:, 3 * CW:4 * CW],
                                            op=OP.mult)
                    r_prev[ch], P_prev[ch] = r_new[ch], P_new[ch]
                    S_last[ch], sigP_last[ch] = S_t[ch], sigP[ch]
                for th in sched.get(t, ()):
                    th()

            # exact final h = tanh(c) * sigma(o) in fp32
            hT = dyn.tile([P, BL], F32, tag="hT")
            for ch in range(NCHAIN):
                nc.vector.tensor_tensor(out=hT[:, ch * CW:(ch + 1) * CW],
                                        in0=sigP_last[ch][:],
                                        in1=S_last[ch][:, 3 * CW:4 * CW],
                                        op=OP.mult)
            nc.sync.dma_start(out=hT_d[:], in_=hT[:])

    nc.compile()
    return nc


def _prep_inputs(text, h_0, emb, conv_w, conv_b, w_ih, w_hh, b_ih, b_hh):
    bf = ml_dtypes.bfloat16
    text = np.asarray(text)
    h_0 = np.asarray(h_0, dtype=np.float32)
    emb = np.asarray(emb, dtype=np.float32)
    conv_w = np.asarray(conv_w, dtype=np.float32)
    conv_b = np.asarray(conv_b, dtype=np.float32)
    w_ih = np.asarray(w_ih, dtype=np.float32)
    w_hh = np.asarray(w_hh, dtype=np.float32)
    b_ih = np.asarray(b_ih, dtype=np.float32)
    b_hh = np.asarray(b_hh, dtype=np.float32)

    emb_bf = np.ascontiguousarray(emb.astype(bf))

    # conv weights: cw[k,eh,nh][e,n] = conv_w[nh*128+n, 0, k, eh*128+e]
    cw = conv_w[:, 0, :, :]                       # [NF, KER, EMB]
    cw = cw.transpose(1, 2, 0)                    # [KER, EMB, NF]
    cw = cw.reshape(KER, 2, P, 2, P)              # k, eh, e, nh, n
    cw = cw.transpose(0, 1, 3, 2, 4)              # k, eh, nh, e, n
    cw_in = np.ascontiguousarray(cw.reshape(KER * 4, P, P).astype(bf))
    cb_in = np.ascontiguousarray(conv_b.reshape(2, P).T)

    # gate reorder torch [i,f,g,o] -> ours [f,i,g,o]
    perm = [1, 0, 2, 3]
    wih_g = w_ih.reshape(4, P, NF)[perm]          # [4, 128, NF]
    whh_g = w_hh.reshape(4, P, HID)[perm]
    bias_g = (b_ih + b_hh).reshape(4, P)[perm]
    wih_g = wih_g * np.array([1, 1, 2, 1], np.float32)[:, None, None]
    bias_g = bias_g * np.array([1, 1, 2, 1], np.float32)[:, None]
    whh_g = whh_g * np.array([1, 1, 2, 1], np.float32)[:, None, None]

    # wih lhsT tiles: [g,kh][k,m] = wih_g[g, m, kh*128+k]
    wih_in = np.ascontiguousarray(
        wih_g.reshape(4, P, 2, P).transpose(0, 2, 3, 1)
        .reshape(8, P, P).astype(bf))
    be_in = np.ascontiguousarray(bias_g.reshape(4, P).T)
    # u lhsT tiles: [g][k,m] = whh_g[g, m, k]
    u_in = np.ascontiguousarray(whh_g.transpose(0, 2, 1).astype(bf))
    eye_in = np.eye(P, dtype=np.float32).astype(bf)

    text32 = text.astype(np.int32)
    in_maps = []
    for cidx in range(NC):
        tloc = text32[cidx * BL:(cidx + 1) * BL]           # [BL, S]
        # idxa[p, q*BL + b] = tloc[b, 127q + p]
        idxa = np.stack([tloc[:, 127 * q:127 * q + P] for q in range(4)],
                        axis=0)                            # [4, BL, 128]
        idxa = np.ascontiguousarray(idxa.transpose(2, 0, 1).reshape(P, BL * 4))
        # idxb[b*4+j, q] = tloc[b, 127q+128+j] (j<3), j=3 dummy
        idxb = np.zeros((BL, 4, 4), np.int32)              # [b, j, q]
        for q in range(4):
            idxb[:, 0:3, q] = tloc[:, 127 * q + 128:127 * q + 131]
        idxb = np.ascontiguousarray(idxb.reshape(P, 4))
        r0 = np.ascontiguousarray(
            h_0[0, cidx * BL:(cidx + 1) * BL].T.astype(bf))
        in_maps.append({
            "embt": emb_bf, "idxa": idxa, "idxb": idxb, "cw": cw_in,
            "cb": cb_in, "wih": wih_in, "be": be_in, "u": u_in,
            "eye": eye_in, "r0": r0,
        })
    return in_maps


def kernel(**inputs) -> np.ndarray:
    global _PROGRAM
    if _PROGRAM is None:
        _PROGRAM = _build_program()
    in_maps = _prep_inputs(**inputs)
    res = run_bass_kernel_spmd(_PROGRAM, in_maps, core_ids=list(range(NC)))
    out = np.empty((B, HID), np.float32)
    for cidx in range(NC):
        out[cidx * BL:(cidx + 1) * BL] = res.results[cidx]["hT"].T
    return out

